# revision 6
# baseline (speedup 1.0000x reference)
"""ALiBi attention (B=2, S=2048, C=1024, H=16) on 8 trn2 NeuronCores.

Sharding: head-parallel. Core c owns heads (c, c+8) for both batches:
  - in_proj computed per-core only for its 6 head-slices (q,k,v x 2 heads),
    directly in transposed [channel, token] layout (x is host-transposed).
  - scores are computed transposed (S^T[j,i] = k_j . q_i) so softmax j-sums
    come from a ones-column augmented onto v, and no transposes of the
    probability matrix are needed.
  - ALiBi bias min(slope*(i-j), 8) is injected into the score PSUM with an
    identity matmul against a host-precomputed shifted bias table.
  - out_proj is row-parallel: each core emits a partial y; the host sums the
    8 partials and adds out_proj_bias (the "all-reduce").
"""
import functools
import math
import sys

sys.path.insert(0, "/opt/trn_rl_repo")

import numpy as np

B, S, C, H, D = 2, 2048, 1024, 16, 64
TOK = B * S
NCORE = 8
MAX_BIAS = 8.0
BTW = 2 * S - 128  # shifted bias-table width
SCALE = float(D) ** -0.5


def _slopes() -> np.ndarray:
    start = 2.0 ** (-(2.0 ** (-(math.log2(H) - 3))))
    return np.array([start * start**i for i in range(H)], dtype=np.float32)


@functools.lru_cache(maxsize=1)
def _program():
    import concourse.mybir as mybir
    import concourse.tile as tile
    from concourse import bacc
    from concourse.masks import make_identity

    F32 = mybir.dt.float32
    F32R = mybir.dt.float32r
    F16 = mybir.dt.float16
    Exp = mybir.ActivationFunctionType.Exp
    MUL = mybir.AluOpType.mult

    nc = bacc.Bacc("TRN2", target_bir_lowering=False, debug=False)

    xt = nc.dram_tensor("xt", [C, TOK], F32R, kind="ExternalInput").ap()
    wqkvt = nc.dram_tensor("wqkvt", [C, 384], F32R, kind="ExternalInput").ap()
    bqkv = nc.dram_tensor("bqkv", [128, 3], F32, kind="ExternalInput").ap()
    bt = nc.dram_tensor("bt", [2, 128, BTW], F32R, kind="ExternalInput").ap()
    wot = nc.dram_tensor("wot", [128, C], F32R, kind="ExternalInput").ap()
    y = nc.dram_tensor("y", [TOK, C], F32, kind="ExternalOutput").ap()

    with tile.TileContext(nc) as tc:
        with tc.tile_pool(name="const", bufs=1) as cpool, \
             tc.tile_pool(name="wpool", bufs=1) as wpool, \
             tc.tile_pool(name="qkvp", bufs=1) as qkvp, \
             tc.tile_pool(name="xin", bufs=3) as xpool, \
             tc.tile_pool(name="probs", bufs=3) as ppool, \
             tc.tile_pool(name="work", bufs=2) as wk, \
             tc.tile_pool(name="ps", bufs=3, space="PSUM") as ps:

            ident = cpool.tile([128, 128], F32, name="ident")
            make_identity(nc, ident[:])
            identr = cpool.tile([128, 128], F32R, name="identr")
            nc.vector.tensor_copy(identr[:], ident[:])
            ones1 = cpool.tile([1, 64], F32, name="ones1")
            nc.vector.memset(ones1[:], 1.0)
            onesr = cpool.tile([1, 64], F32R, name="onesr")
            nc.vector.tensor_copy(onesr[:], ones1[:])
            neg8 = cpool.tile([128, 1], F32, name="neg8")
            nc.vector.memset(neg8[:], -MAX_BIAS)

            wq_sb = wpool.tile([128, 8, 384], F32R, name="wq_sb")
            nc.sync.dma_start(wq_sb[:], wqkvt.rearrange("(co p) n -> p co n", p=128))
            bq_sb = wpool.tile([128, 3], F32, name="bq_sb")
            nc.sync.dma_start(bq_sb[:], bqkv)
            btab = wpool.tile([128, 2, BTW], F32R, name="btab")
            nc.sync.dma_start(btab[:], bt.rearrange("h p c -> p h c"))
            wo_sb = wpool.tile([128, C], F32R, name="wo_sb")
            nc.sync.dma_start(wo_sb[:], wot)

            qkvT = qkvp.tile([128, 3, TOK], F32R, name="qkvT")
            v_nat = qkvp.tile([128, 32, 2, 65], F16, name="v_nat")
            nc.vector.memset(v_nat[:, :, :, 64:65], 1.0)
            oT = qkvp.tile([128, TOK], F32R, name="oT")

            xt_r = xt.rearrange("(co p) t -> p co t", p=128)
            y_r = y.rearrange("(tb p) c -> tb p c", p=128)

            # ---- in_proj: qkvT[ch, tok] = W_slice @ x^T + b ----
            scope_inproj = nc.named_scope("inproj"); scope_inproj.__enter__()
            for tb in range(8):
                xtile = xpool.tile([128, 8, 512], F32R, name=f"xt{tb}", tag="xtile")
                nc.sync.dma_start(xtile[:], xt_r[:, :, tb * 512:(tb + 1) * 512])
                for chb in range(3):
                    pin = ps.tile([128, 512], F32, name=f"pin{tb}_{chb}", tag="sc")
                    for cb in range(8):
                        nc.tensor.matmul(
                            pin[:],
                            wq_sb[:, cb, chb * 128:(chb + 1) * 128],
                            xtile[:, cb, :],
                            start=(cb == 0), stop=(cb == 7),
                        )
                    nc.vector.tensor_scalar_add(
                        qkvT[:, chb, tb * 512:(tb + 1) * 512], pin[:],
                        bq_sb[:, chb:chb + 1],
                    )

            scope_inproj.__exit__(None, None, None)
            # ---- v -> natural [token, d] layout (fp16), with ones column ----
            scope_vt = nc.named_scope("vt"); scope_vt.__enter__()
            for t32 in range(32):
                pv = ps.tile([128, 128], F32, name=f"pv{t32}", tag="sc")
                nc.tensor.transpose(
                    pv[:], qkvT[:, 2, t32 * 128:(t32 + 1) * 128].bitcast(F32),
                    ident[:],
                )
                for hh in range(2):
                    nc.vector.tensor_copy(
                        v_nat[:, t32, hh, 0:64], pv[:, hh * 64:hh * 64 + 64]
                    )

            scope_vt.__exit__(None, None, None)
            scope_attn = nc.named_scope("attn"); scope_attn.__enter__()
            # ---- attention (scores transposed; flash-free full-row softmax) ----
            # Tile classification for slot-0 heads (heads 0..7 across cores;
            # identical on every core, so the SPMD program stays uniform):
            #   - skip:  far-future tiles, prob mass < ~1e-7 of the row sum
            #   - fold:  bias saturated at +8 everywhere -> skip the bias
            #            inject and use exp bias 0 instead of -8
            zero0 = cpool.tile([128, 1], F32, name="zero0")
            nc.vector.memset(zero0[:], 0.0)
            BF16 = mybir.dt.bfloat16
            heat = cpool.tile([128, 64], BF16, name="heat")
            nc.vector.tensor_copy(heat[:], ident[:, 0:64])
            for b in range(2):
                for hh in range(2):
                    hb = hh * 64
                    for ih in range(2):
                        i0 = ih * 1024
                        it = f"{b}{hh}{ih}"
                        js = []
                        for j in range(16):
                            j0 = j * 128
                            if hh == 0 and j0 - i0 >= 1483:
                                continue  # negligible far-future tile
                            js.append(j)
                        pacc = ps.tile([65, 1024], F32, name=f"pa{it}", tag="acc", bufs=1)
                        pending = None  # (pb_tile, j, is_first)
                        for idx, j in enumerate(js):
                            j0 = j * 128
                            fold = hh == 0 and i0 - j0 >= 255
                            pS = ps.tile([128, 1024], F32, name=f"pS{it}_{j}",
                                         tag="sc")
                            # HAM heater: bf16 MM whose result is discarded by
                            # the start=True QKT below (keeps PE clock at 8/8)
                            nc.tensor.matmul(pS[0:64, 0:64], heat[:], heat[:],
                                             start=True, stop=True,
                                             skip_group_check=True)
                            kT = qkvT[hb:hb + 64, 1,
                                      b * 2048 + j0: b * 2048 + j0 + 128]
                            for iq in range(2):
                                ii = i0 + iq * 512
                                qT = qkvT[hb:hb + 64, 0,
                                          b * 2048 + ii: b * 2048 + ii + 512]
                                nc.tensor.matmul(pS[:, iq * 512:(iq + 1) * 512],
                                                 kT, qT, start=True, stop=fold)
                                if not fold:
                                    c0 = ii - j0 + (S - 128)
                                    nc.tensor.matmul(
                                        pS[:, iq * 512:(iq + 1) * 512],
                                        identr[:], btab[:, hh, c0:c0 + 512],
                                        start=False, stop=True)
                            pb = ppool.tile([128, 1024], F16, name=f"pb{it}_{j}",
                                            tag="pb")
                            nc.scalar.activation(
                                pb[:], pS[:], Exp,
                                bias=(zero0 if fold else neg8)[:, 0:1], scale=1.0)
                            if pending is not None:
                                pvb, pvj, pvfirst = pending
                                for iq in range(2):
                                    nc.tensor.matmul(
                                        pacc[:, iq * 512:(iq + 1) * 512],
                                        v_nat[:, b * 16 + pvj, hh, :],
                                        pvb[:, iq * 512:(iq + 1) * 512],
                                        start=pvfirst, stop=False)
                            pending = (pb, j, idx == 0)
                        pvb, pvj, pvfirst = pending
                        for iq in range(2):
                            nc.tensor.matmul(
                                pacc[:, iq * 512:(iq + 1) * 512],
                                v_nat[:, b * 16 + pvj, hh, :],
                                pvb[:, iq * 512:(iq + 1) * 512],
                                start=pvfirst, stop=True)
                        # normalization: oT = pacc[0:64] * (1/rowsum) bcast.
                        # Reciprocal in [8, 128] layout (8 lanes, cheap); the
                        # row<->col reshapes ride on otherwise-idle DMA queues.
                        sumr = wk.tile([1, 1024], F32, name=f"sr{it}", tag="sumr", bufs=1)
                        nc.vector.tensor_copy(sumr[:], pacc[64:65, :])
                        sumc = wk.tile([8, 128], F32, name=f"sc{it}", tag="sumc")
                        nc.sync.dma_start(
                            sumc[:],
                            sumr[:].rearrange("o (p a) -> o p a", a=128))
                        inv8 = wk.tile([8, 128], F32R, name=f"i8{it}", tag="inv8")
                        with nc.allow_low_precision(reason="f32r bcast"):
                            nc.vector.reciprocal(inv8[:], sumc[:])
                        invr = wk.tile([1, 1024], F32R, name=f"iv{it}", tag="invr", bufs=1)
                        nc.sync.dma_start(
                            invr[:].rearrange("o (p a) -> o p a", a=128),
                            inv8[:])
                        pB = ps.tile([64, 1024], F32, name=f"pB{it}", tag="sc")
                        for iq in range(2):
                            nc.tensor.matmul(pB[:, iq * 512:(iq + 1) * 512],
                                             onesr[:],
                                             invr[:, iq * 512:(iq + 1) * 512],
                                             start=True, stop=True)
                        osl = oT[hb:hb + 64, b * 2048 + i0: b * 2048 + i0 + 1024]
                        with nc.allow_low_precision(reason="f32r out"):
                            nc.vector.tensor_copy(osl, pacc[0:64, :])
                            nc.vector.tensor_tensor(osl, osl, pB[:], MUL)

            scope_attn.__exit__(None, None, None)
            scope_op = nc.named_scope("outproj"); scope_op.__enter__()
            for tb in range(32):
                py_ = ps.tile([128, 1024], F32, name=f"py{tb}", tag="sc")
                for cq in range(2):
                    nc.tensor.matmul(py_[:, cq * 512:(cq + 1) * 512],
                                     oT[:, tb * 128:(tb + 1) * 128],
                                     wo_sb[:, cq * 512:(cq + 1) * 512],
                                     start=True, stop=True)
                ytile = wk.tile([128, 1024], F32, name=f"yt{tb}", tag="ytile")
                nc.vector.tensor_copy(ytile[:], py_[:])
                nc.sync.dma_start(y_r[tb], ytile[:])
            scope_op.__exit__(None, None, None)

    nc.compile()
    return nc


@functools.lru_cache(maxsize=1)
def _host_prep_cache():
    return {}


def _make_inmaps(x, in_proj_weight, in_proj_bias, out_proj_weight):
    slopes = _slopes()
    xT = np.ascontiguousarray(
        x.reshape(TOK, C).T.astype(np.float32))  # [C, TOK]

    in_maps = []
    p = np.arange(128, dtype=np.float64)[:, None]
    cc = np.arange(BTW, dtype=np.float64)[None, :]
    for c in range(NCORE):
        heads = (c, c + 8)
        rows = []
        for sec in range(3):  # q, k, v
            for h in heads:
                rows.extend(range(sec * C + h * D, sec * C + (h + 1) * D))
        rows = np.array(rows)
        wq = in_proj_weight[rows, :].astype(np.float32)
        bq = in_proj_bias[rows].astype(np.float32).copy()
        wq = wq.copy()
        wq[:128] *= SCALE  # fold q scaling
        bq[:128] *= SCALE
        wqkvt = np.ascontiguousarray(wq.T)  # [C, 384]
        bqkv = np.ascontiguousarray(bq.reshape(3, 128).T)  # [128, 3]

        btarr = np.empty((2, 128, BTW), dtype=np.float32)
        for hh, h in enumerate(heads):
            btarr[hh] = np.minimum(
                float(slopes[h]) * (cc - (S - 128) - p), float(MAX_BIAS)
            ).astype(np.float32)

        ocols = np.array(
            [heads[0] * D + d for d in range(D)]
            + [heads[1] * D + d for d in range(D)]
        )
        wotr = np.ascontiguousarray(
            out_proj_weight[:, ocols].T.astype(np.float32))  # [128, C]

        in_maps.append({
            "xt": xT,
            "wqkvt": wqkvt,
            "bqkv": bqkv,
            "bt": btarr,
            "wot": wotr,
        })
    return in_maps


def run(inputs: dict, trace: bool = False):
    from concourse.bass_utils import run_bass_kernel_spmd

    nc = _program()
    in_maps = _make_inmaps(
        np.asarray(inputs["x"]),
        np.asarray(inputs["in_proj_weight"]),
        np.asarray(inputs["in_proj_bias"]),
        np.asarray(inputs["out_proj_weight"]),
    )
    res = run_bass_kernel_spmd(nc, in_maps, list(range(NCORE)), trace=trace)
    acc = np.zeros((TOK, C), dtype=np.float64)
    for r in res.results:
        acc += r["y"].astype(np.float64)
    acc += np.asarray(inputs["out_proj_bias"]).astype(np.float64)[None, :]
    out = acc.astype(np.float32).reshape(B, S, C)
    return out, res


def kernel(**inputs) -> np.ndarray:
    return run(inputs, trace=False)[0]


# revision 8
# speedup vs baseline: 1.4264x; 1.4264x over previous
"""ALiBi attention (B=2, S=2048, C=1024, H=16) on 8 trn2 NeuronCores.

Sharding: head-parallel. Core c owns heads (c, c+8) for both batches:
  - in_proj computed per-core only for its 6 head-slices (q,k,v x 2 heads),
    directly in transposed [channel, token] layout (x is host-transposed).
  - scores are computed transposed (S^T[j,i] = k_j . q_i) so softmax j-sums
    come from a ones-column augmented onto v, and no transposes of the
    probability matrix are needed.
  - ALiBi bias min(slope*(i-j), 8) is injected into the score PSUM with an
    identity matmul against a host-precomputed shifted bias table.
  - out_proj is row-parallel: each core emits a partial y; the host sums the
    8 partials and adds out_proj_bias (the "all-reduce").
"""
import functools
import math
import sys

sys.path.insert(0, "/opt/trn_rl_repo")

import numpy as np

B, S, C, H, D = 2, 2048, 1024, 16, 64
TOK = B * S
NCORE = 8
MAX_BIAS = 8.0
BTW = 2 * S - 128  # shifted bias-table width
SCALE = float(D) ** -0.5


def _slopes() -> np.ndarray:
    start = 2.0 ** (-(2.0 ** (-(math.log2(H) - 3))))
    return np.array([start * start**i for i in range(H)], dtype=np.float32)


@functools.lru_cache(maxsize=1)
def _program():
    import concourse.mybir as mybir
    import concourse.tile as tile
    from concourse import bacc
    from concourse.masks import make_identity

    F32 = mybir.dt.float32
    F32R = mybir.dt.float32r
    F16 = mybir.dt.float16
    Exp = mybir.ActivationFunctionType.Exp
    MUL = mybir.AluOpType.mult

    nc = bacc.Bacc("TRN2", target_bir_lowering=False, debug=False)

    xt = nc.dram_tensor("xt", [C, TOK], F32R, kind="ExternalInput").ap()
    wqkvt = nc.dram_tensor("wqkvt", [C, 384], F32R, kind="ExternalInput").ap()
    bqkv = nc.dram_tensor("bqkv", [128, 3], F32, kind="ExternalInput").ap()
    bt = nc.dram_tensor("bt", [2, 128, BTW], F32R, kind="ExternalInput").ap()
    wot = nc.dram_tensor("wot", [128, C], F32R, kind="ExternalInput").ap()
    y = nc.dram_tensor("y", [TOK, C], F32, kind="ExternalOutput").ap()

    with tile.TileContext(nc) as tc:
        with tc.tile_pool(name="const", bufs=1) as cpool, \
             tc.tile_pool(name="wpool", bufs=1) as wpool, \
             tc.tile_pool(name="qkvp", bufs=1) as qkvp, \
             tc.tile_pool(name="xin", bufs=2) as xpool, \
             tc.tile_pool(name="probs", bufs=2) as ppool, \
             tc.tile_pool(name="work", bufs=2) as wk, \
             tc.tile_pool(name="ps", bufs=3, space="PSUM") as ps:

            ident = cpool.tile([128, 128], F32, name="ident")
            make_identity(nc, ident[:])
            identr = cpool.tile([128, 128], F32R, name="identr")
            nc.vector.tensor_copy(identr[:], ident[:])
            ones1 = cpool.tile([1, 64], F32, name="ones1")
            nc.vector.memset(ones1[:], 1.0)
            onesr = cpool.tile([1, 64], F32R, name="onesr")
            nc.vector.tensor_copy(onesr[:], ones1[:])
            neg8 = cpool.tile([128, 1], F32, name="neg8")
            nc.vector.memset(neg8[:], -MAX_BIAS)

            wq_sb = wpool.tile([128, 8, 384], F32R, name="wq_sb")
            nc.sync.dma_start(wq_sb[:], wqkvt.rearrange("(co p) n -> p co n", p=128))
            bq_sb = wpool.tile([128, 3], F32, name="bq_sb")
            nc.sync.dma_start(bq_sb[:], bqkv)
            btab = wpool.tile([128, 2, BTW], F32R, name="btab")
            nc.sync.dma_start(btab[:], bt.rearrange("h p c -> p h c"))
            wo_sb = wpool.tile([128, C], F32R, name="wo_sb")
            nc.sync.dma_start(wo_sb[:], wot)

            qkvT = qkvp.tile([128, 3, TOK], F32R, name="qkvT")
            # k stationaries are zero-padded to K=128 per head (rows of the
            # other head are 0) so every attention matmul keeps the same
            # (128,128) PE row/col configuration -- mixed K=64/K=128 f32r
            # streams run ~3x slower. qkvT[:,1] holds head-A k (rows 64-127
            # zero); kpadB holds head-B k (rows 0-63 zero).
            kpadB = qkvp.tile([128, TOK], F32R, name="kpadB")
            U32 = mybir.dt.uint32
            nc.vector.memset(qkvT[64:128, 1, :].bitcast(U32), 0)
            nc.vector.memset(kpadB[0:64, :].bitcast(U32), 0)
            v_nat = qkvp.tile([128, 32, 2, 65], F16, name="v_nat")
            nc.vector.memset(v_nat[:, :, :, 64:65], 1.0)
            oT = qkvp.tile([128, TOK], F32R, name="oT")

            xt_r = xt.rearrange("(co p) t -> p co t", p=128)
            y_r = y.rearrange("(tb p) c -> tb p c", p=128)

            # ---- in_proj: qkvT[ch, tok] = W_slice @ x^T + b ----
            scope_inproj = nc.named_scope("inproj"); scope_inproj.__enter__()
            for tb in range(8):
                xtile = xpool.tile([128, 8, 512], F32R, name=f"xt{tb}", tag="xtile")
                nc.sync.dma_start(xtile[:], xt_r[:, :, tb * 512:(tb + 1) * 512])
                for chb in range(3):
                    pin = ps.tile([128, 512], F32, name=f"pin{tb}_{chb}", tag="sc")
                    for cb in range(8):
                        nc.tensor.matmul(
                            pin[:],
                            wq_sb[:, cb, chb * 128:(chb + 1) * 128],
                            xtile[:, cb, :],
                            start=(cb == 0), stop=(cb == 7),
                        )
                    if chb == 1:
                        nc.vector.tensor_scalar_add(
                            qkvT[0:64, 1, tb * 512:(tb + 1) * 512], pin[0:64],
                            bq_sb[0:64, 1:2])
                        nc.vector.tensor_scalar_add(
                            kpadB[64:128, tb * 512:(tb + 1) * 512], pin[64:128],
                            bq_sb[64:128, 1:2])
                    else:
                        nc.vector.tensor_scalar_add(
                            qkvT[:, chb, tb * 512:(tb + 1) * 512], pin[:],
                            bq_sb[:, chb:chb + 1],
                        )

            scope_inproj.__exit__(None, None, None)
            # ---- v -> natural [token, d] layout (fp16), with ones column ----
            scope_vt = nc.named_scope("vt"); scope_vt.__enter__()
            for t32 in range(32):
                pv = ps.tile([128, 128], F32, name=f"pv{t32}", tag="sc")
                nc.tensor.transpose(
                    pv[:], qkvT[:, 2, t32 * 128:(t32 + 1) * 128].bitcast(F32),
                    ident[:],
                )
                for hh in range(2):
                    nc.vector.tensor_copy(
                        v_nat[:, t32, hh, 0:64], pv[:, hh * 64:hh * 64 + 64]
                    )

            scope_vt.__exit__(None, None, None)
            scope_attn = nc.named_scope("attn"); scope_attn.__enter__()
            # ---- attention (scores transposed; flash-free full-row softmax) ----
            # Tile classification for slot-0 heads (heads 0..7 across cores;
            # identical on every core, so the SPMD program stays uniform):
            #   - skip:  far-future tiles, prob mass < ~1e-7 of the row sum
            #   - fold:  bias saturated at +8 everywhere -> skip the bias
            #            inject and use exp bias 0 instead of -8
            zero0 = cpool.tile([128, 1], F32, name="zero0")
            nc.vector.memset(zero0[:], 0.0)
            BF16 = mybir.dt.bfloat16
            heat = cpool.tile([128, 128], BF16, name="heat")
            nc.vector.tensor_copy(heat[:], ident[:])
            for b in range(2):
                for hh in range(2):
                    hb = hh * 64
                    for ih in range(2):
                        i0 = ih * 1024
                        it = f"{b}{hh}{ih}"
                        js = []
                        for j in range(16):
                            j0 = j * 128
                            if hh == 0 and j0 - i0 >= 1483:
                                continue  # negligible far-future tile
                            js.append(j)
                        pacc = ps.tile([65, 1024], F32, name=f"pa{it}", tag="acc", bufs=1)
                        pending = None  # (pb_tile, j, is_first)
                        for idx, j in enumerate(js):
                            j0 = j * 128
                            fold = hh == 0 and i0 - j0 >= 255
                            pS = ps.tile([128, 1024], F32, name=f"pS{it}_{j}",
                                         tag="sc")
                            # HAM heater: bf16 MM whose result is discarded by
                            # the start=True QKT below (keeps PE clock at 8/8)
                            nc.tensor.matmul(pS[:, 0:128], heat[:], heat[:],
                                             start=True, stop=True,
                                             skip_group_check=True)
                            if hh == 0:
                                kT = qkvT[:, 1,
                                          b * 2048 + j0: b * 2048 + j0 + 128]
                            else:
                                kT = kpadB[:, b * 2048 + j0: b * 2048 + j0 + 128]
                            for iq in range(2):
                                ii = i0 + iq * 512
                                qT = qkvT[:, 0,
                                          b * 2048 + ii: b * 2048 + ii + 512]
                                nc.tensor.matmul(pS[:, iq * 512:(iq + 1) * 512],
                                                 kT, qT, start=True, stop=fold)
                                if not fold:
                                    c0 = ii - j0 + (S - 128)
                                    nc.tensor.matmul(
                                        pS[:, iq * 512:(iq + 1) * 512],
                                        identr[:], btab[:, hh, c0:c0 + 512],
                                        start=False, stop=True)
                            pb = ppool.tile([128, 1024], F16, name=f"pb{it}_{j}",
                                            tag="pb")
                            nc.scalar.activation(
                                pb[:], pS[:], Exp,
                                bias=(zero0 if fold else neg8)[:, 0:1], scale=1.0)
                            if pending is not None:
                                pvb, pvj, pvfirst = pending
                                for iq in range(2):
                                    nc.tensor.matmul(
                                        pacc[:, iq * 512:(iq + 1) * 512],
                                        v_nat[:, b * 16 + pvj, hh, :],
                                        pvb[:, iq * 512:(iq + 1) * 512],
                                        start=pvfirst, stop=False)
                            pending = (pb, j, idx == 0)
                        pvb, pvj, pvfirst = pending
                        for iq in range(2):
                            nc.tensor.matmul(
                                pacc[:, iq * 512:(iq + 1) * 512],
                                v_nat[:, b * 16 + pvj, hh, :],
                                pvb[:, iq * 512:(iq + 1) * 512],
                                start=pvfirst, stop=True)
                        # normalization: oT = pacc[0:64] * (1/rowsum) bcast.
                        # Reciprocal in [8, 128] layout (8 lanes, cheap); the
                        # row<->col reshapes ride on otherwise-idle DMA queues.
                        sumr = wk.tile([1, 1024], F32, name=f"sr{it}", tag="sumr", bufs=1)
                        nc.vector.tensor_copy(sumr[:], pacc[64:65, :])
                        sumc = wk.tile([8, 128], F32, name=f"sc{it}", tag="sumc")
                        nc.sync.dma_start(
                            sumc[:],
                            sumr[:].rearrange("o (p a) -> o p a", a=128))
                        inv8 = wk.tile([8, 128], F32R, name=f"i8{it}", tag="inv8")
                        with nc.allow_low_precision(reason="f32r bcast"):
                            nc.vector.reciprocal(inv8[:], sumc[:])
                        invr = wk.tile([1, 1024], F32R, name=f"iv{it}", tag="invr", bufs=1)
                        nc.sync.dma_start(
                            invr[:].rearrange("o (p a) -> o p a", a=128),
                            inv8[:])
                        pB = ps.tile([64, 1024], F32, name=f"pB{it}", tag="sc")
                        for iq in range(2):
                            nc.tensor.matmul(pB[:, iq * 512:(iq + 1) * 512],
                                             onesr[:],
                                             invr[:, iq * 512:(iq + 1) * 512],
                                             start=True, stop=True)
                        osl = oT[hb:hb + 64, b * 2048 + i0: b * 2048 + i0 + 1024]
                        with nc.allow_low_precision(reason="f32r out"):
                            nc.vector.tensor_copy(osl, pacc[0:64, :])
                            nc.vector.tensor_tensor(osl, osl, pB[:], MUL)

            scope_attn.__exit__(None, None, None)
            scope_op = nc.named_scope("outproj"); scope_op.__enter__()
            for tb in range(32):
                py_ = ps.tile([128, 1024], F32, name=f"py{tb}", tag="sc")
                for cq in range(2):
                    nc.tensor.matmul(py_[:, cq * 512:(cq + 1) * 512],
                                     oT[:, tb * 128:(tb + 1) * 128],
                                     wo_sb[:, cq * 512:(cq + 1) * 512],
                                     start=True, stop=True)
                for cq in range(2):
                    ytile = wk.tile([128, 512], F32, name=f"yt{tb}_{cq}",
                                    tag="ytile")
                    nc.vector.tensor_copy(ytile[:],
                                          py_[:, cq * 512:(cq + 1) * 512])
                    nc.sync.dma_start(y_r[tb][:, cq * 512:(cq + 1) * 512],
                                      ytile[:])
            scope_op.__exit__(None, None, None)

    nc.compile()
    return nc


@functools.lru_cache(maxsize=1)
def _host_prep_cache():
    return {}


def _make_inmaps(x, in_proj_weight, in_proj_bias, out_proj_weight):
    slopes = _slopes()
    xT = np.ascontiguousarray(
        x.reshape(TOK, C).T.astype(np.float32))  # [C, TOK]

    in_maps = []
    p = np.arange(128, dtype=np.float64)[:, None]
    cc = np.arange(BTW, dtype=np.float64)[None, :]
    for c in range(NCORE):
        heads = (c, c + 8)
        rows = []
        for sec in range(3):  # q, k, v
            for h in heads:
                rows.extend(range(sec * C + h * D, sec * C + (h + 1) * D))
        rows = np.array(rows)
        wq = in_proj_weight[rows, :].astype(np.float32)
        bq = in_proj_bias[rows].astype(np.float32).copy()
        wq = wq.copy()
        wq[:128] *= SCALE  # fold q scaling
        bq[:128] *= SCALE
        wqkvt = np.ascontiguousarray(wq.T)  # [C, 384]
        bqkv = np.ascontiguousarray(bq.reshape(3, 128).T)  # [128, 3]

        btarr = np.empty((2, 128, BTW), dtype=np.float32)
        for hh, h in enumerate(heads):
            btarr[hh] = np.minimum(
                float(slopes[h]) * (cc - (S - 128) - p), float(MAX_BIAS)
            ).astype(np.float32)

        ocols = np.array(
            [heads[0] * D + d for d in range(D)]
            + [heads[1] * D + d for d in range(D)]
        )
        wotr = np.ascontiguousarray(
            out_proj_weight[:, ocols].T.astype(np.float32))  # [128, C]

        in_maps.append({
            "xt": xT,
            "wqkvt": wqkvt,
            "bqkv": bqkv,
            "bt": btarr,
            "wot": wotr,
        })
    return in_maps


def run(inputs: dict, trace: bool = False):
    from concourse.bass_utils import run_bass_kernel_spmd

    nc = _program()
    in_maps = _make_inmaps(
        np.asarray(inputs["x"]),
        np.asarray(inputs["in_proj_weight"]),
        np.asarray(inputs["in_proj_bias"]),
        np.asarray(inputs["out_proj_weight"]),
    )
    res = run_bass_kernel_spmd(nc, in_maps, list(range(NCORE)), trace=trace)
    acc = np.zeros((TOK, C), dtype=np.float64)
    for r in res.results:
        acc += r["y"].astype(np.float64)
    acc += np.asarray(inputs["out_proj_bias"]).astype(np.float64)[None, :]
    out = acc.astype(np.float32).reshape(B, S, C)
    return out, res


def kernel(**inputs) -> np.ndarray:
    return run(inputs, trace=False)[0]


# revision 10
# speedup vs baseline: 1.6916x; 1.1859x over previous
"""ALiBi attention (B=2, S=2048, C=1024, H=16) on 8 trn2 NeuronCores.

Sharding: head-parallel. Core c owns heads (c, c+8) for both batches:
  - in_proj computed per-core only for its 6 head-slices (q,k,v x 2 heads),
    directly in transposed [channel, token] layout (x is host-transposed).
  - scores are computed transposed (S^T[j,i] = k_j . q_i) so softmax j-sums
    come from a ones-column augmented onto v, and the probability matrix is
    never transposed.
  - ALiBi bias min(slope*(i-j), 8) is injected into the score PSUM with an
    identity matmul against a host-precomputed shifted bias table; tiles where
    the bias is saturated at +8 skip the inject (the +8 cancels against the
    exp's -8 range shift), and far-future tiles with negligible probability
    mass are skipped entirely. Both classifications depend only on the head
    SLOT (slot 0 = heads 0..7, slot 1 = heads 8..15), so the single SPMD
    program stays valid on every core.
  - k stationaries are zero-padded to K=128 per head (the other head's rows
    are 0, killing its q rows in the shared moving operand): mixed K=64/K=128
    f32r matmul streams reconfigure the PE array and run ~3x slower.
  - out_proj is row-parallel: each core emits a partial y; the host sums the
    8 partials and adds out_proj_bias (the "all-reduce").
"""
import functools
import math
import sys

sys.path.insert(0, "/opt/trn_rl_repo")

import numpy as np

B, S, C, H, D = 2, 2048, 1024, 16, 64
TOK = B * S
NCORE = 8
MAX_BIAS = 8.0
BTW = 2 * S - 128       # shifted bias-table width (full, for slot-1 heads)
BT0_OFF = 384           # slot-0 table column offset (unfolded tiles only)
BT0_W = 2816            # slot-0 table width
SCALE = float(D) ** -0.5
SKIP_J_MINUS_I = 1483   # skip tile if j0 - i0 >= this (slot 0 only)
FOLD_I_MINUS_J = 255    # inject-free tile if i0 - j0 >= this (slot 0 only)


def _slopes() -> np.ndarray:
    start = 2.0 ** (-(2.0 ** (-(math.log2(H) - 3))))
    return np.array([start * start**i for i in range(H)], dtype=np.float32)


@functools.lru_cache(maxsize=1)
def _program():
    import concourse.mybir as mybir
    import concourse.tile as tile
    from concourse import bacc
    from concourse.masks import make_identity

    F32 = mybir.dt.float32
    F32R = mybir.dt.float32r
    F16 = mybir.dt.float16
    BF16 = mybir.dt.bfloat16
    U32 = mybir.dt.uint32
    Exp = mybir.ActivationFunctionType.Exp
    MUL = mybir.AluOpType.mult

    nc = bacc.Bacc("TRN2", target_bir_lowering=False, debug=False)

    xt = nc.dram_tensor("xt", [C, TOK], F32R, kind="ExternalInput").ap()
    wqkvt = nc.dram_tensor("wqkvt", [C, 384], F32R, kind="ExternalInput").ap()
    bqkv = nc.dram_tensor("bqkv", [128, 3], F32, kind="ExternalInput").ap()
    bt = nc.dram_tensor("bt", [2, 128, BTW], F32R, kind="ExternalInput").ap()
    wot = nc.dram_tensor("wot", [128, C], F32R, kind="ExternalInput").ap()
    y = nc.dram_tensor("y", [TOK, C], F32, kind="ExternalOutput").ap()

    with tile.TileContext(nc) as tc:
        with tc.tile_pool(name="const", bufs=1) as cpool, \
             tc.tile_pool(name="wpool", bufs=1) as wpool, \
             tc.tile_pool(name="qkvp", bufs=1) as qkvp, \
             tc.tile_pool(name="xin", bufs=2) as xpool, \
             tc.tile_pool(name="probs", bufs=2) as ppool, \
             tc.tile_pool(name="work", bufs=2) as wk, \
             tc.tile_pool(name="ps", bufs=2, space="PSUM") as ps:

            ident = cpool.tile([128, 128], F32, name="ident")
            make_identity(nc, ident[:])
            identr = cpool.tile([128, 128], F32R, name="identr")
            nc.vector.tensor_copy(identr[:], ident[:])
            neg8 = cpool.tile([128, 1], F32, name="neg8")
            nc.vector.memset(neg8[:], -MAX_BIAS)
            zero0 = cpool.tile([128, 1], F32, name="zero0")
            nc.vector.memset(zero0[:], 0.0)
            heat = cpool.tile([128, 128], BF16, name="heat")
            nc.vector.tensor_copy(heat[:], ident[:])

            wq_sb = wpool.tile([128, 8, 384], F32R, name="wq_sb")
            nc.sync.dma_start(wq_sb[:],
                              wqkvt.rearrange("(co p) n -> p co n", p=128))
            bq_sb = wpool.tile([128, 3], F32, name="bq_sb")
            nc.sync.dma_start(bq_sb[:], bqkv)
            btab1 = wpool.tile([128, BTW], F32R, name="btab1")
            nc.sync.dma_start(btab1[:], bt.rearrange("h p c -> p h c")[:, 1, :])
            btab0 = wpool.tile([128, BT0_W], F32R, name="btab0")
            nc.sync.dma_start(
                btab0[:],
                bt.rearrange("h p c -> p h c")[:, 0, BT0_OFF:BT0_OFF + BT0_W])
            wo_sb = wpool.tile([128, C], F32R, name="wo_sb")
            nc.sync.dma_start(wo_sb[:], wot)

            qkvT = qkvp.tile([128, 3, TOK], F32R, name="qkvT")
            kpadB = qkvp.tile([128, TOK], F32R, name="kpadB")
            nc.vector.memset(qkvT[64:128, 1, :].bitcast(U32), 0)
            nc.vector.memset(kpadB[0:64, :].bitcast(U32), 0)
            v_nat = qkvp.tile([128, 32, 2, 65], F16, name="v_nat")
            nc.vector.memset(v_nat[:, :, :, 64:65], 1.0)
            oT = qkvp.tile([128, TOK], F32R, name="oT")

            xt_r = xt.rearrange("(co p) t -> p co t", p=128)
            y_r = y.rearrange("(tb p) c -> tb p c", p=128)

            def in_proj(bb):
                for tb in range(4 * bb, 4 * bb + 4):
                    xtile = xpool.tile([128, 8, 512], F32R, name=f"xt{tb}",
                                       tag="xtile")
                    nc.sync.dma_start(xtile[:],
                                      xt_r[:, :, tb * 512:(tb + 1) * 512])
                    for chb in range(3):
                        pin = ps.tile([128, 512], F32, name=f"pin{tb}_{chb}",
                                      tag="sc")
                        for cb in range(8):
                            nc.tensor.matmul(
                                pin[:],
                                wq_sb[:, cb, chb * 128:(chb + 1) * 128],
                                xtile[:, cb, :],
                                start=(cb == 0), stop=(cb == 7))
                        ts = slice(tb * 512, (tb + 1) * 512)
                        if chb == 1:
                            nc.vector.tensor_scalar_add(
                                qkvT[0:64, 1, ts], pin[0:64], bq_sb[0:64, 1:2])
                            nc.vector.tensor_scalar_add(
                                kpadB[64:128, ts], pin[64:128],
                                bq_sb[64:128, 1:2])
                        else:
                            nc.vector.tensor_scalar_add(
                                qkvT[:, chb, ts], pin[:], bq_sb[:, chb:chb + 1])

            def v_transpose(bb):
                for t32 in range(16 * bb, 16 * bb + 16):
                    pv = ps.tile([128, 128], F32, name=f"pv{t32}", tag="sc")
                    nc.tensor.transpose(
                        pv[:],
                        qkvT[:, 2, t32 * 128:(t32 + 1) * 128].bitcast(F32),
                        ident[:])
                    for hh in range(2):
                        nc.vector.tensor_copy(v_nat[:, t32, hh, 0:64],
                                              pv[:, hh * 64:hh * 64 + 64])

            def attn_iter(b, ih, hh):
                hb = hh * 64
                i0 = ih * 1024
                it = f"{b}{ih}{hh}"
                js = [j for j in range(16)
                      if not (hh == 0 and j * 128 - i0 >= SKIP_J_MINUS_I)]
                pacc = ps.tile([65, 1024], F32, name=f"pa{it}", tag="acc")
                pending = None
                for idx, j in enumerate(js):
                    j0 = j * 128
                    fold = hh == 0 and i0 - j0 >= FOLD_I_MINUS_J
                    pS = ps.tile([128, 1024], F32, name=f"pS{it}_{j}", tag="sc")
                    nc.tensor.matmul(pS[:, 0:128], heat[:], heat[:],
                                     start=True, stop=True,
                                     skip_group_check=True)
                    if hh == 0:
                        kT = qkvT[:, 1, b * 2048 + j0: b * 2048 + j0 + 128]
                    else:
                        kT = kpadB[:, b * 2048 + j0: b * 2048 + j0 + 128]
                    for iq in range(2):
                        ii = i0 + iq * 512
                        sl = pS[:, iq * 512:(iq + 1) * 512]
                        qT = qkvT[:, 0, b * 2048 + ii: b * 2048 + ii + 512]
                        nc.tensor.matmul(sl, kT, qT, start=True, stop=fold)
                        if not fold:
                            c0 = ii - j0 + (S - 128)
                            if hh == 0:
                                rhs = btab0[:, c0 - BT0_OFF:c0 - BT0_OFF + 512]
                            else:
                                rhs = btab1[:, c0:c0 + 512]
                            nc.tensor.matmul(sl, identr[:], rhs,
                                             start=False, stop=True)
                    pb = ppool.tile([128, 1024], F16, name=f"pb{it}_{j}",
                                    tag="pb")
                    nc.scalar.activation(pb[:], pS[:], Exp,
                                         bias=(zero0 if fold else neg8)[:, 0:1],
                                         scale=1.0)
                    if pending is not None:
                        pvb, pvj, first = pending
                        for iq in range(2):
                            nc.tensor.matmul(pacc[:, iq * 512:(iq + 1) * 512],
                                             v_nat[:, b * 16 + pvj, hh, :],
                                             pvb[:, iq * 512:(iq + 1) * 512],
                                             start=first, stop=False)
                    pending = (pb, j, idx == 0)
                pvb, pvj, first = pending
                for iq in range(2):
                    nc.tensor.matmul(pacc[:, iq * 512:(iq + 1) * 512],
                                     v_nat[:, b * 16 + pvj, hh, :],
                                     pvb[:, iq * 512:(iq + 1) * 512],
                                     start=first, stop=True)
                # normalization: oT = pacc[0:64] * (1/rowsum).
                # reciprocal runs in [8,128] layout (cheap); row<->col reshapes
                # ride on DMA; the broadcast runs on the idle GpSimd engine.
                sumr = wk.tile([1, 1024], F32, name=f"sr{it}", tag="sumr",
                               bufs=1)
                nc.vector.tensor_copy(sumr[:], pacc[64:65, :])
                sumc = wk.tile([8, 128], F32, name=f"sc{it}", tag="sumc")
                nc.sync.dma_start(sumc[:],
                                  sumr[:].rearrange("o (p a) -> o p a", a=128))
                inv8 = wk.tile([8, 128], F32, name=f"i8{it}", tag="inv8")
                nc.vector.reciprocal(inv8[:], sumc[:])
                invr = wk.tile([1, 1024], F32, name=f"iv{it}", tag="invr",
                               bufs=1)
                nc.sync.dma_start(invr[:].rearrange("o (p a) -> o p a", a=128),
                                  inv8[:])
                invbc = wk.tile([128, 1024], F32, name=f"ib{it}", tag="invbc",
                                bufs=1)
                nc.gpsimd.partition_broadcast(invbc[:], invr[:], channels=128)
                osl = oT[hb:hb + 64, b * 2048 + i0: b * 2048 + i0 + 1024]
                with nc.allow_low_precision(reason="f32r out"):
                    nc.vector.tensor_copy(osl, pacc[0:64, :])
                    nc.vector.tensor_tensor(osl, osl, invbc[hb:hb + 64, :], MUL)

            def out_proj(b, ih):
                for tloc in range(8):
                    tb = b * 16 + ih * 8 + tloc
                    py_ = ps.tile([128, 1024], F32, name=f"py{tb}", tag="sc")
                    for cq in range(2):
                        nc.tensor.matmul(py_[:, cq * 512:(cq + 1) * 512],
                                         oT[:, tb * 128:(tb + 1) * 128],
                                         wo_sb[:, cq * 512:(cq + 1) * 512],
                                         start=True, stop=True)
                    for cq in range(2):
                        ytile = wk.tile([128, 512], F32, name=f"yt{tb}_{cq}",
                                        tag="ytile")
                        nc.vector.tensor_copy(ytile[:],
                                              py_[:, cq * 512:(cq + 1) * 512])
                        nc.sync.dma_start(y_r[tb][:, cq * 512:(cq + 1) * 512],
                                          ytile[:])

            units = [(0, 0), (0, 1), (1, 0), (1, 1)]
            for k, (b, ih) in enumerate(units):
                if (b, ih) == (0, 0):
                    in_proj(0)
                    v_transpose(0)
                if (b, ih) == (1, 0):
                    in_proj(1)
                    v_transpose(1)
                if k >= 2:
                    out_proj(*units[k - 2])
                for hh in range(2):
                    attn_iter(b, ih, hh)
            out_proj(*units[2])
            out_proj(*units[3])

    nc.compile()
    return nc


def _make_inmaps(x, in_proj_weight, in_proj_bias, out_proj_weight):
    slopes = _slopes()
    xT = np.ascontiguousarray(
        x.reshape(TOK, C).T.astype(np.float32))  # [C, TOK]

    in_maps = []
    p = np.arange(128, dtype=np.float64)[:, None]
    cc = np.arange(BTW, dtype=np.float64)[None, :]
    for c in range(NCORE):
        heads = (c, c + 8)
        rows = []
        for sec in range(3):  # q, k, v
            for h in heads:
                rows.extend(range(sec * C + h * D, sec * C + (h + 1) * D))
        rows = np.array(rows)
        wq = in_proj_weight[rows, :].astype(np.float32).copy()
        bq = in_proj_bias[rows].astype(np.float32).copy()
        wq[:128] *= SCALE  # fold q scaling
        bq[:128] *= SCALE
        wqkvt = np.ascontiguousarray(wq.T)  # [C, 384]
        bqkv = np.ascontiguousarray(bq.reshape(3, 128).T)  # [128, 3]

        btarr = np.empty((2, 128, BTW), dtype=np.float32)
        for hh, h in enumerate(heads):
            btarr[hh] = np.minimum(
                float(slopes[h]) * (cc - (S - 128) - p), float(MAX_BIAS)
            ).astype(np.float32)

        ocols = np.array(
            [heads[0] * D + d for d in range(D)]
            + [heads[1] * D + d for d in range(D)]
        )
        wotr = np.ascontiguousarray(
            out_proj_weight[:, ocols].T.astype(np.float32))  # [128, C]

        in_maps.append({
            "xt": xT,
            "wqkvt": wqkvt,
            "bqkv": bqkv,
            "bt": btarr,
            "wot": wotr,
        })
    return in_maps


def run(inputs: dict, trace: bool = False):
    from concourse.bass_utils import run_bass_kernel_spmd

    nc = _program()
    in_maps = _make_inmaps(
        np.asarray(inputs["x"]),
        np.asarray(inputs["in_proj_weight"]),
        np.asarray(inputs["in_proj_bias"]),
        np.asarray(inputs["out_proj_weight"]),
    )
    res = run_bass_kernel_spmd(nc, in_maps, list(range(NCORE)), trace=trace)
    acc = np.zeros((TOK, C), dtype=np.float64)
    for r in res.results:
        acc += r["y"].astype(np.float64)
    acc += np.asarray(inputs["out_proj_bias"]).astype(np.float64)[None, :]
    out = acc.astype(np.float32).reshape(B, S, C)
    return out, res


def kernel(**inputs) -> np.ndarray:
    return run(inputs, trace=False)[0]


# revision 11
# speedup vs baseline: 1.6973x; 1.0033x over previous
"""ALiBi attention (B=2, S=2048, C=1024, H=16) on 8 trn2 NeuronCores.

Sharding: head-parallel. Core c owns heads (c, c+8) for both batches:
  - in_proj computed per-core only for its 6 head-slices (q,k,v x 2 heads),
    directly in transposed [channel, token] layout (x is host-transposed).
  - scores are computed transposed (S^T[j,i] = k_j . q_i) so softmax j-sums
    come from a ones-column augmented onto v, and the probability matrix is
    never transposed.
  - ALiBi bias min(slope*(i-j), 8) is injected into the score PSUM with an
    identity matmul against a host-precomputed shifted bias table; tiles where
    the bias is saturated at +8 skip the inject (the +8 cancels against the
    exp's -8 range shift), and far-future tiles with negligible probability
    mass are skipped entirely. Both classifications depend only on the head
    SLOT (slot 0 = heads 0..7, slot 1 = heads 8..15), so the single SPMD
    program stays valid on every core.
  - k stationaries are zero-padded to K=128 per head (the other head's rows
    are 0, killing its q rows in the shared moving operand): mixed K=64/K=128
    f32r matmul streams reconfigure the PE array and run ~3x slower.
  - out_proj is row-parallel: each core emits a partial y; the host sums the
    8 partials and adds out_proj_bias (the "all-reduce").
"""
import functools
import math
import sys

sys.path.insert(0, "/opt/trn_rl_repo")

import numpy as np

B, S, C, H, D = 2, 2048, 1024, 16, 64
TOK = B * S
NCORE = 8
MAX_BIAS = 8.0
BTW = 2 * S - 128       # shifted bias-table width (full, for slot-1 heads)
BT0_OFF = 384           # slot-0 table column offset (unfolded tiles only)
BT0_W = 2816            # slot-0 table width
SCALE = float(D) ** -0.5
SKIP_J_MINUS_I = 1483   # skip tile if j0 - i0 >= this (slot 0 only)
FOLD_I_MINUS_J = 255    # inject-free tile if i0 - j0 >= this (slot 0 only)


def _slopes() -> np.ndarray:
    start = 2.0 ** (-(2.0 ** (-(math.log2(H) - 3))))
    return np.array([start * start**i for i in range(H)], dtype=np.float32)


@functools.lru_cache(maxsize=1)
def _program():
    import concourse.mybir as mybir
    import concourse.tile as tile
    from concourse import bacc
    from concourse.masks import make_identity

    F32 = mybir.dt.float32
    F32R = mybir.dt.float32r
    F16 = mybir.dt.float16
    BF16 = mybir.dt.bfloat16
    U32 = mybir.dt.uint32
    Exp = mybir.ActivationFunctionType.Exp
    MUL = mybir.AluOpType.mult

    nc = bacc.Bacc("TRN2", target_bir_lowering=False, debug=False)

    xt = nc.dram_tensor("xt", [C, TOK], F32R, kind="ExternalInput").ap()
    wqkvt = nc.dram_tensor("wqkvt", [C, 384], F32R, kind="ExternalInput").ap()
    bqkv = nc.dram_tensor("bqkv", [128, 3], F32, kind="ExternalInput").ap()
    bt = nc.dram_tensor("bt", [2, 128, BTW], F32R, kind="ExternalInput").ap()
    wot = nc.dram_tensor("wot", [128, C], F32R, kind="ExternalInput").ap()
    y = nc.dram_tensor("y", [TOK, C], F32, kind="ExternalOutput").ap()

    with tile.TileContext(nc) as tc:
        with tc.tile_pool(name="const", bufs=1) as cpool, \
             tc.tile_pool(name="wpool", bufs=1) as wpool, \
             tc.tile_pool(name="qkvp", bufs=1) as qkvp, \
             tc.tile_pool(name="xin", bufs=2) as xpool, \
             tc.tile_pool(name="probs", bufs=2) as ppool, \
             tc.tile_pool(name="work", bufs=2) as wk, \
             tc.tile_pool(name="ps", bufs=2, space="PSUM") as ps:

            ident = cpool.tile([128, 128], F32, name="ident")
            make_identity(nc, ident[:])
            identr = cpool.tile([128, 128], F32R, name="identr")
            nc.vector.tensor_copy(identr[:], ident[:])
            neg8 = cpool.tile([128, 1], F32, name="neg8")
            nc.vector.memset(neg8[:], -MAX_BIAS)
            zero0 = cpool.tile([128, 1], F32, name="zero0")
            nc.vector.memset(zero0[:], 0.0)
            heat = cpool.tile([128, 128], BF16, name="heat")
            nc.vector.tensor_copy(heat[:], ident[:])

            wq_sb = wpool.tile([128, 8, 384], F32R, name="wq_sb")
            nc.sync.dma_start(wq_sb[:],
                              wqkvt.rearrange("(co p) n -> p co n", p=128))
            bq_sb = wpool.tile([128, 3], F32, name="bq_sb")
            nc.sync.dma_start(bq_sb[:], bqkv)
            btab1 = wpool.tile([128, BTW], F32R, name="btab1")
            btab0 = wpool.tile([128, BT0_W], F32R, name="btab0")
            wo_sb = wpool.tile([128, C], F32R, name="wo_sb")

            def load_tables():
                nc.sync.dma_start(btab1[:],
                                  bt.rearrange("h p c -> p h c")[:, 1, :])
                nc.sync.dma_start(
                    btab0[:],
                    bt.rearrange("h p c -> p h c")[:, 0,
                                                   BT0_OFF:BT0_OFF + BT0_W])
                nc.sync.dma_start(wo_sb[:], wot)

            qkvT = qkvp.tile([128, 3, TOK], F32R, name="qkvT")
            kpadB = qkvp.tile([128, TOK], F32R, name="kpadB")
            nc.vector.memset(qkvT[64:128, 1, :].bitcast(U32), 0)
            nc.vector.memset(kpadB[0:64, :].bitcast(U32), 0)
            v_nat = qkvp.tile([128, 32, 2, 65], F16, name="v_nat")
            nc.vector.memset(v_nat[:, :, :, 64:65], 1.0)
            oT = qkvp.tile([128, TOK], F32R, name="oT")

            xt_r = xt.rearrange("(co p) t -> p co t", p=128)
            y_r = y.rearrange("(tb p) c -> tb p c", p=128)

            def in_proj(bb):
                for tb in range(4 * bb, 4 * bb + 4):
                    xtile = xpool.tile([128, 8, 512], F32R, name=f"xt{tb}",
                                       tag="xtile")
                    nc.sync.dma_start(xtile[:],
                                      xt_r[:, :, tb * 512:(tb + 1) * 512])
                    for chb in range(3):
                        pin = ps.tile([128, 512], F32, name=f"pin{tb}_{chb}",
                                      tag="sc")
                        for cb in range(8):
                            nc.tensor.matmul(
                                pin[:],
                                wq_sb[:, cb, chb * 128:(chb + 1) * 128],
                                xtile[:, cb, :],
                                start=(cb == 0), stop=(cb == 7))
                        ts = slice(tb * 512, (tb + 1) * 512)
                        if chb == 1:
                            nc.vector.tensor_scalar_add(
                                qkvT[0:64, 1, ts], pin[0:64], bq_sb[0:64, 1:2])
                            nc.vector.tensor_scalar_add(
                                kpadB[64:128, ts], pin[64:128],
                                bq_sb[64:128, 1:2])
                        else:
                            nc.vector.tensor_scalar_add(
                                qkvT[:, chb, ts], pin[:], bq_sb[:, chb:chb + 1])

            def v_transpose(bb):
                for t32 in range(16 * bb, 16 * bb + 16):
                    pv = ps.tile([128, 128], F32, name=f"pv{t32}", tag="sc")
                    nc.tensor.transpose(
                        pv[:],
                        qkvT[:, 2, t32 * 128:(t32 + 1) * 128].bitcast(F32),
                        ident[:])
                    for hh in range(2):
                        nc.vector.tensor_copy(v_nat[:, t32, hh, 0:64],
                                              pv[:, hh * 64:hh * 64 + 64])

            def attn_iter(b, ih, hh):
                hb = hh * 64
                i0 = ih * 1024
                it = f"{b}{ih}{hh}"
                js = [j for j in range(16)
                      if not (hh == 0 and j * 128 - i0 >= SKIP_J_MINUS_I)]
                pacc = ps.tile([65, 1024], F32, name=f"pa{it}", tag="acc")
                pending = None
                for idx, j in enumerate(js):
                    j0 = j * 128
                    fold = hh == 0 and i0 - j0 >= FOLD_I_MINUS_J
                    pS = ps.tile([128, 1024], F32, name=f"pS{it}_{j}", tag="sc")
                    nc.tensor.matmul(pS[:, 0:128], heat[:], heat[:],
                                     start=True, stop=True,
                                     skip_group_check=True)
                    if hh == 0:
                        kT = qkvT[:, 1, b * 2048 + j0: b * 2048 + j0 + 128]
                    else:
                        kT = kpadB[:, b * 2048 + j0: b * 2048 + j0 + 128]
                    for iq in range(2):
                        ii = i0 + iq * 512
                        sl = pS[:, iq * 512:(iq + 1) * 512]
                        qT = qkvT[:, 0, b * 2048 + ii: b * 2048 + ii + 512]
                        nc.tensor.matmul(sl, kT, qT, start=True, stop=fold)
                        if not fold:
                            c0 = ii - j0 + (S - 128)
                            if hh == 0:
                                rhs = btab0[:, c0 - BT0_OFF:c0 - BT0_OFF + 512]
                            else:
                                rhs = btab1[:, c0:c0 + 512]
                            nc.tensor.matmul(sl, identr[:], rhs,
                                             start=False, stop=True)
                    pb = ppool.tile([128, 1024], F16, name=f"pb{it}_{j}",
                                    tag="pb")
                    nc.scalar.activation(pb[:], pS[:], Exp,
                                         bias=(zero0 if fold else neg8)[:, 0:1],
                                         scale=1.0)
                    if pending is not None:
                        pvb, pvj, first = pending
                        for iq in range(2):
                            nc.tensor.matmul(pacc[:, iq * 512:(iq + 1) * 512],
                                             v_nat[:, b * 16 + pvj, hh, :],
                                             pvb[:, iq * 512:(iq + 1) * 512],
                                             start=first, stop=False)
                    pending = (pb, j, idx == 0)
                pvb, pvj, first = pending
                for iq in range(2):
                    nc.tensor.matmul(pacc[:, iq * 512:(iq + 1) * 512],
                                     v_nat[:, b * 16 + pvj, hh, :],
                                     pvb[:, iq * 512:(iq + 1) * 512],
                                     start=first, stop=True)
                # normalization: oT = pacc[0:64] * (1/rowsum).
                # reciprocal runs in [8,128] layout (cheap); row<->col reshapes
                # ride on DMA; the broadcast runs on the idle GpSimd engine.
                sumr = wk.tile([1, 1024], F32, name=f"sr{it}", tag="sumr",
                               bufs=1)
                nc.vector.tensor_copy(sumr[:], pacc[64:65, :])
                sumc = wk.tile([8, 128], F32, name=f"sc{it}", tag="sumc")
                nc.sync.dma_start(sumc[:],
                                  sumr[:].rearrange("o (p a) -> o p a", a=128))
                inv8 = wk.tile([8, 128], F32, name=f"i8{it}", tag="inv8")
                nc.vector.reciprocal(inv8[:], sumc[:])
                invr = wk.tile([1, 1024], F32, name=f"iv{it}", tag="invr",
                               bufs=1)
                nc.sync.dma_start(invr[:].rearrange("o (p a) -> o p a", a=128),
                                  inv8[:])
                invbc = wk.tile([128, 1024], F32, name=f"ib{it}", tag="invbc",
                                bufs=1)
                nc.gpsimd.partition_broadcast(invbc[:], invr[:], channels=128)
                osl = oT[hb:hb + 64, b * 2048 + i0: b * 2048 + i0 + 1024]
                with nc.allow_low_precision(reason="f32r out"):
                    nc.vector.tensor_copy(osl, pacc[0:64, :])
                    nc.vector.tensor_tensor(osl, osl, invbc[hb:hb + 64, :], MUL)

            def out_proj(b, ih):
                for tloc in range(8):
                    tb = b * 16 + ih * 8 + tloc
                    py_ = ps.tile([128, 1024], F32, name=f"py{tb}", tag="sc")
                    for cq in range(2):
                        nc.tensor.matmul(py_[:, cq * 512:(cq + 1) * 512],
                                         oT[:, tb * 128:(tb + 1) * 128],
                                         wo_sb[:, cq * 512:(cq + 1) * 512],
                                         start=True, stop=True)
                    for cq in range(2):
                        ytile = wk.tile([128, 512], F32, name=f"yt{tb}_{cq}",
                                        tag="ytile")
                        if cq == 0:
                            nc.vector.tensor_copy(
                                ytile[:], py_[:, cq * 512:(cq + 1) * 512])
                        else:
                            nc.scalar.copy(
                                ytile[:], py_[:, cq * 512:(cq + 1) * 512])
                        nc.sync.dma_start(y_r[tb][:, cq * 512:(cq + 1) * 512],
                                          ytile[:])

            in_proj(0)
            load_tables()
            v_transpose(0)
            attn_iter(0, 0, 0)
            attn_iter(0, 0, 1)
            attn_iter(0, 1, 0)
            attn_iter(0, 1, 1)
            in_proj(1)
            v_transpose(1)
            attn_iter(1, 0, 0)
            out_proj(0, 0)
            attn_iter(1, 0, 1)
            out_proj(0, 1)
            attn_iter(1, 1, 0)
            out_proj(1, 0)
            attn_iter(1, 1, 1)
            out_proj(1, 1)

    nc.compile()
    return nc


def _make_inmaps(x, in_proj_weight, in_proj_bias, out_proj_weight):
    slopes = _slopes()
    xT = np.ascontiguousarray(
        x.reshape(TOK, C).T.astype(np.float32))  # [C, TOK]

    in_maps = []
    p = np.arange(128, dtype=np.float64)[:, None]
    cc = np.arange(BTW, dtype=np.float64)[None, :]
    for c in range(NCORE):
        heads = (c, c + 8)
        rows = []
        for sec in range(3):  # q, k, v
            for h in heads:
                rows.extend(range(sec * C + h * D, sec * C + (h + 1) * D))
        rows = np.array(rows)
        wq = in_proj_weight[rows, :].astype(np.float32).copy()
        bq = in_proj_bias[rows].astype(np.float32).copy()
        wq[:128] *= SCALE  # fold q scaling
        bq[:128] *= SCALE
        wqkvt = np.ascontiguousarray(wq.T)  # [C, 384]
        bqkv = np.ascontiguousarray(bq.reshape(3, 128).T)  # [128, 3]

        btarr = np.empty((2, 128, BTW), dtype=np.float32)
        for hh, h in enumerate(heads):
            btarr[hh] = np.minimum(
                float(slopes[h]) * (cc - (S - 128) - p), float(MAX_BIAS)
            ).astype(np.float32)

        ocols = np.array(
            [heads[0] * D + d for d in range(D)]
            + [heads[1] * D + d for d in range(D)]
        )
        wotr = np.ascontiguousarray(
            out_proj_weight[:, ocols].T.astype(np.float32))  # [128, C]

        in_maps.append({
            "xt": xT,
            "wqkvt": wqkvt,
            "bqkv": bqkv,
            "bt": btarr,
            "wot": wotr,
        })
    return in_maps


def run(inputs: dict, trace: bool = False):
    from concourse.bass_utils import run_bass_kernel_spmd

    nc = _program()
    in_maps = _make_inmaps(
        np.asarray(inputs["x"]),
        np.asarray(inputs["in_proj_weight"]),
        np.asarray(inputs["in_proj_bias"]),
        np.asarray(inputs["out_proj_weight"]),
    )
    res = run_bass_kernel_spmd(nc, in_maps, list(range(NCORE)), trace=trace)
    acc = np.zeros((TOK, C), dtype=np.float64)
    for r in res.results:
        acc += r["y"].astype(np.float64)
    acc += np.asarray(inputs["out_proj_bias"]).astype(np.float64)[None, :]
    out = acc.astype(np.float32).reshape(B, S, C)
    return out, res


def kernel(**inputs) -> np.ndarray:
    return run(inputs, trace=False)[0]


# revision 12
# speedup vs baseline: 2.0331x; 1.1979x over previous
"""ALiBi attention (B=2, S=2048, C=1024, H=16) on 8 trn2 NeuronCores.

Sharding: head-parallel. Core c owns heads (c, c+8) for both batches:
  - in_proj computed per-core only for its 6 head-slices (q,k,v x 2 heads),
    directly in transposed [channel, token] layout (x is host-transposed).
  - scores are computed transposed (S^T[j,i] = k_j . q_i) so softmax j-sums
    come from a ones-column augmented onto v, and the probability matrix is
    never transposed.
  - ALiBi bias min(slope*(i-j), 8) is injected into the score PSUM with an
    identity matmul against a host-precomputed shifted bias table; tiles where
    the bias is saturated at +8 skip the inject (the +8 cancels against the
    exp's -8 range shift), and far-future tiles with negligible probability
    mass are skipped entirely. Both classifications depend only on the head
    SLOT (slot 0 = heads 0..7, slot 1 = heads 8..15), so the single SPMD
    program stays valid on every core.
  - k stationaries are zero-padded to K=128 per head (the other head's rows
    are 0, killing its q rows in the shared moving operand): mixed K=64/K=128
    f32r matmul streams reconfigure the PE array and run ~3x slower.
  - out_proj is row-parallel: each core emits a partial y; the host sums the
    8 partials and adds out_proj_bias (the "all-reduce").
"""
import functools
import math
import sys

sys.path.insert(0, "/opt/trn_rl_repo")

import numpy as np

B, S, C, H, D = 2, 2048, 1024, 16, 64
TOK = B * S
NCORE = 8
MAX_BIAS = 8.0
BTW = 2 * S - 128       # shifted bias-table width (full, for slot-1 heads)
BT0_OFF = 384           # slot-0 table column offset (unfolded tiles only)
BT0_W = 2816            # slot-0 table width
SCALE = float(D) ** -0.5
SKIP_J_MINUS_I = 1483   # skip tile if j0 - i0 >= this (slot 0 only)
FOLD_I_MINUS_J = 255    # inject-free tile if i0 - j0 >= this (slot 0 only)


def _slopes() -> np.ndarray:
    start = 2.0 ** (-(2.0 ** (-(math.log2(H) - 3))))
    return np.array([start * start**i for i in range(H)], dtype=np.float32)


@functools.lru_cache(maxsize=1)
def _program():
    import concourse.mybir as mybir
    import concourse.tile as tile
    from concourse import bacc
    from concourse.masks import make_identity

    F32 = mybir.dt.float32
    F32R = mybir.dt.float32r
    F16 = mybir.dt.float16
    BF16 = mybir.dt.bfloat16
    U32 = mybir.dt.uint32
    Exp = mybir.ActivationFunctionType.Exp
    MUL = mybir.AluOpType.mult

    nc = bacc.Bacc("TRN2", target_bir_lowering=False, debug=False)

    xt = nc.dram_tensor("xt", [C, TOK], F32R, kind="ExternalInput").ap()
    wqkvt = nc.dram_tensor("wqkvt", [C, 384], F32R, kind="ExternalInput").ap()
    bqkv = nc.dram_tensor("bqkv", [128, 3], F32, kind="ExternalInput").ap()
    bt = nc.dram_tensor("bt", [2, 128, BTW], F32R, kind="ExternalInput").ap()
    wot = nc.dram_tensor("wot", [128, C], F32R, kind="ExternalInput").ap()
    y = nc.dram_tensor("y", [TOK, C], F32, kind="ExternalOutput").ap()

    with tile.TileContext(nc) as tc:
        with tc.tile_pool(name="const", bufs=1) as cpool, \
             tc.tile_pool(name="wpool", bufs=1) as wpool, \
             tc.tile_pool(name="qkvp", bufs=1) as qkvp, \
             tc.tile_pool(name="xin", bufs=2) as xpool, \
             tc.tile_pool(name="probs", bufs=2) as ppool, \
             tc.tile_pool(name="work", bufs=2) as wk, \
             tc.tile_pool(name="ps", bufs=2, space="PSUM") as ps:

            ident = cpool.tile([128, 128], F32, name="ident")
            make_identity(nc, ident[:])
            identr = cpool.tile([128, 128], F32R, name="identr")
            nc.vector.tensor_copy(identr[:], ident[:])
            neg8 = cpool.tile([128, 1], F32, name="neg8")
            nc.vector.memset(neg8[:], -MAX_BIAS)
            zero0 = cpool.tile([128, 1], F32, name="zero0")
            nc.vector.memset(zero0[:], 0.0)
            heat = cpool.tile([128, 128], BF16, name="heat")
            nc.vector.tensor_copy(heat[:], ident[:])

            wq_sb = wpool.tile([128, 8, 384], F32R, name="wq_sb")
            nc.sync.dma_start(wq_sb[:],
                              wqkvt.rearrange("(co p) n -> p co n", p=128))
            bq_sb = wpool.tile([128, 3], F32, name="bq_sb")
            nc.sync.dma_start(bq_sb[:], bqkv)
            btab1 = wpool.tile([128, BTW], F32R, name="btab1")
            btab0 = wpool.tile([128, BT0_W], F32R, name="btab0")
            wo_sb = wpool.tile([128, C], F32R, name="wo_sb")

            def load_tables():
                nc.sync.dma_start(btab1[:],
                                  bt.rearrange("h p c -> p h c")[:, 1, :])
                nc.sync.dma_start(
                    btab0[:],
                    bt.rearrange("h p c -> p h c")[:, 0,
                                                   BT0_OFF:BT0_OFF + BT0_W])
                nc.sync.dma_start(wo_sb[:], wot)

            qkvT = qkvp.tile([128, 3, TOK], F32R, name="qkvT")
            kpadB = qkvp.tile([128, TOK], F32R, name="kpadB")
            nc.vector.memset(qkvT[64:128, 1, :].bitcast(U32), 0)
            nc.vector.memset(kpadB[0:64, :].bitcast(U32), 0)
            v_nat = qkvp.tile([128, 32, 2, 65], F16, name="v_nat")
            nc.vector.memset(v_nat[:, :, :, 64:65], 1.0)
            oT = qkvp.tile([128, TOK], F32R, name="oT")

            xt_r = xt.rearrange("(co p) t -> p co t", p=128)
            y_r = y.rearrange("(tb p) c -> tb p c", p=128)

            def in_proj(bb):
                for tb in range(4 * bb, 4 * bb + 4):
                    xtile = xpool.tile([128, 8, 512], F32R, name=f"xt{tb}",
                                       tag="xtile")
                    nc.sync.dma_start(xtile[:],
                                      xt_r[:, :, tb * 512:(tb + 1) * 512])
                    for chb in range(3):
                        pin = ps.tile([128, 512], F32, name=f"pin{tb}_{chb}",
                                      tag="sc")
                        for cb in range(8):
                            nc.tensor.matmul(
                                pin[:],
                                wq_sb[:, cb, chb * 128:(chb + 1) * 128],
                                xtile[:, cb, :],
                                start=(cb == 0), stop=(cb == 7))
                        ts = slice(tb * 512, (tb + 1) * 512)
                        if chb == 1:
                            nc.vector.tensor_scalar_add(
                                qkvT[0:64, 1, ts], pin[0:64], bq_sb[0:64, 1:2])
                            nc.vector.tensor_scalar_add(
                                kpadB[64:128, ts], pin[64:128],
                                bq_sb[64:128, 1:2])
                        else:
                            nc.vector.tensor_scalar_add(
                                qkvT[:, chb, ts], pin[:], bq_sb[:, chb:chb + 1])

            def v_transpose(bb):
                for t32 in range(16 * bb, 16 * bb + 16):
                    pv = ps.tile([128, 128], F32, name=f"pv{t32}", tag="sc")
                    nc.tensor.transpose(
                        pv[:],
                        qkvT[:, 2, t32 * 128:(t32 + 1) * 128].bitcast(F32),
                        ident[:])
                    for hh in range(2):
                        nc.vector.tensor_copy(v_nat[:, t32, hh, 0:64],
                                              pv[:, hh * 64:hh * 64 + 64])

            def attn_iter(b, ih, hh):
                hb = hh * 64
                i0 = ih * 1024
                it = f"{b}{ih}{hh}"
                js = [j for j in range(16)
                      if not (hh == 0 and j * 128 - i0 >= SKIP_J_MINUS_I)]
                pacc = ps.tile([65, 1024], F32, name=f"pa{it}", tag="acc",
                               bufs=1)
                pending = None
                for idx, j in enumerate(js):
                    j0 = j * 128
                    fold = hh == 0 and i0 - j0 >= FOLD_I_MINUS_J
                    pS = ps.tile([128, 1024], F32, name=f"pS{it}_{j}", tag="sc")
                    nc.tensor.matmul(pS[:, 0:128], heat[:], heat[:],
                                     start=True, stop=True,
                                     skip_group_check=True)
                    if hh == 0:
                        kT = qkvT[:, 1, b * 2048 + j0: b * 2048 + j0 + 128]
                    else:
                        kT = kpadB[:, b * 2048 + j0: b * 2048 + j0 + 128]
                    for iq in range(2):
                        ii = i0 + iq * 512
                        sl = pS[:, iq * 512:(iq + 1) * 512]
                        qT = qkvT[:, 0, b * 2048 + ii: b * 2048 + ii + 512]
                        nc.tensor.matmul(sl, kT, qT, start=True, stop=fold)
                        if not fold:
                            c0 = ii - j0 + (S - 128)
                            if hh == 0:
                                rhs = btab0[:, c0 - BT0_OFF:c0 - BT0_OFF + 512]
                            else:
                                rhs = btab1[:, c0:c0 + 512]
                            nc.tensor.matmul(sl, identr[:], rhs,
                                             start=False, stop=True)
                    pb = ppool.tile([128, 1024], F16, name=f"pb{it}_{j}",
                                    tag="pb")
                    nc.scalar.activation(pb[:], pS[:], Exp,
                                         bias=(zero0 if fold else neg8)[:, 0:1],
                                         scale=1.0)
                    if pending is not None:
                        pvb, pvj, first = pending
                        for iq in range(2):
                            nc.tensor.matmul(pacc[:, iq * 512:(iq + 1) * 512],
                                             v_nat[:, b * 16 + pvj, hh, :],
                                             pvb[:, iq * 512:(iq + 1) * 512],
                                             start=first, stop=False)
                    pending = (pb, j, idx == 0)
                pvb, pvj, first = pending
                for iq in range(2):
                    nc.tensor.matmul(pacc[:, iq * 512:(iq + 1) * 512],
                                     v_nat[:, b * 16 + pvj, hh, :],
                                     pvb[:, iq * 512:(iq + 1) * 512],
                                     start=first, stop=True)
                # normalization: oT = pacc[0:64] * (1/rowsum).
                # reciprocal runs in [8,128] layout (cheap); row<->col reshapes
                # ride on DMA; the broadcast runs on the idle GpSimd engine.
                sumr = wk.tile([1, 1024], F32, name=f"sr{it}", tag="sumr",
                               bufs=1)
                nc.vector.tensor_copy(sumr[:], pacc[64:65, :])
                sumc = wk.tile([8, 128], F32, name=f"sc{it}", tag="sumc")
                nc.sync.dma_start(sumc[:],
                                  sumr[:].rearrange("o (p a) -> o p a", a=128))
                inv8 = wk.tile([8, 128], F32, name=f"i8{it}", tag="inv8")
                nc.vector.reciprocal(inv8[:], sumc[:])
                invr = wk.tile([1, 1024], F32, name=f"iv{it}", tag="invr",
                               bufs=1)
                nc.sync.dma_start(invr[:].rearrange("o (p a) -> o p a", a=128),
                                  inv8[:])
                invbc = wk.tile([128, 1024], F32, name=f"ib{it}", tag="invbc",
                                bufs=1)
                nc.gpsimd.partition_broadcast(invbc[:], invr[:], channels=128)
                osl = oT[hb:hb + 64, b * 2048 + i0: b * 2048 + i0 + 1024]
                with nc.allow_low_precision(reason="f32r out"):
                    nc.vector.tensor_copy(osl, pacc[0:64, :])
                    nc.vector.tensor_tensor(osl, osl, invbc[hb:hb + 64, :], MUL)

            def out_proj(b, ih):
                for tloc in range(8):
                    tb = b * 16 + ih * 8 + tloc
                    for cq in range(2):
                        py_ = ps.tile([128, 512], F32, name=f"py{tb}_{cq}",
                                      tag="py")
                        nc.tensor.matmul(py_[:],
                                         oT[:, tb * 128:(tb + 1) * 128],
                                         wo_sb[:, cq * 512:(cq + 1) * 512],
                                         start=True, stop=True)
                        ytile = wk.tile([128, 512], F32, name=f"yt{tb}_{cq}",
                                        tag="ytile")
                        if cq == 0:
                            nc.vector.tensor_copy(ytile[:], py_[:])
                        else:
                            nc.scalar.copy(ytile[:], py_[:])
                        nc.sync.dma_start(y_r[tb][:, cq * 512:(cq + 1) * 512],
                                          ytile[:])

            in_proj(0)
            load_tables()
            v_transpose(0)
            attn_iter(0, 0, 0)
            attn_iter(0, 0, 1)
            attn_iter(0, 1, 0)
            attn_iter(0, 1, 1)
            in_proj(1)
            v_transpose(1)
            attn_iter(1, 0, 0)
            out_proj(0, 0)
            attn_iter(1, 0, 1)
            out_proj(0, 1)
            attn_iter(1, 1, 0)
            out_proj(1, 0)
            attn_iter(1, 1, 1)
            out_proj(1, 1)

    nc.compile()
    return nc


def _make_inmaps(x, in_proj_weight, in_proj_bias, out_proj_weight):
    slopes = _slopes()
    xT = np.ascontiguousarray(
        x.reshape(TOK, C).T.astype(np.float32))  # [C, TOK]

    in_maps = []
    p = np.arange(128, dtype=np.float64)[:, None]
    cc = np.arange(BTW, dtype=np.float64)[None, :]
    for c in range(NCORE):
        heads = (c, c + 8)
        rows = []
        for sec in range(3):  # q, k, v
            for h in heads:
                rows.extend(range(sec * C + h * D, sec * C + (h + 1) * D))
        rows = np.array(rows)
        wq = in_proj_weight[rows, :].astype(np.float32).copy()
        bq = in_proj_bias[rows].astype(np.float32).copy()
        wq[:128] *= SCALE  # fold q scaling
        bq[:128] *= SCALE
        wqkvt = np.ascontiguousarray(wq.T)  # [C, 384]
        bqkv = np.ascontiguousarray(bq.reshape(3, 128).T)  # [128, 3]

        btarr = np.empty((2, 128, BTW), dtype=np.float32)
        for hh, h in enumerate(heads):
            btarr[hh] = np.minimum(
                float(slopes[h]) * (cc - (S - 128) - p), float(MAX_BIAS)
            ).astype(np.float32)

        ocols = np.array(
            [heads[0] * D + d for d in range(D)]
            + [heads[1] * D + d for d in range(D)]
        )
        wotr = np.ascontiguousarray(
            out_proj_weight[:, ocols].T.astype(np.float32))  # [128, C]

        in_maps.append({
            "xt": xT,
            "wqkvt": wqkvt,
            "bqkv": bqkv,
            "bt": btarr,
            "wot": wotr,
        })
    return in_maps


def run(inputs: dict, trace: bool = False):
    from concourse.bass_utils import run_bass_kernel_spmd

    nc = _program()
    in_maps = _make_inmaps(
        np.asarray(inputs["x"]),
        np.asarray(inputs["in_proj_weight"]),
        np.asarray(inputs["in_proj_bias"]),
        np.asarray(inputs["out_proj_weight"]),
    )
    res = run_bass_kernel_spmd(nc, in_maps, list(range(NCORE)), trace=trace)
    acc = np.zeros((TOK, C), dtype=np.float64)
    for r in res.results:
        acc += r["y"].astype(np.float64)
    acc += np.asarray(inputs["out_proj_bias"]).astype(np.float64)[None, :]
    out = acc.astype(np.float32).reshape(B, S, C)
    return out, res


def kernel(**inputs) -> np.ndarray:
    return run(inputs, trace=False)[0]
